# revision 35
# baseline (speedup 1.0000x reference)
"""Trainium2 Bass kernel for nn_BgeAttention (dense transformer block).

Sharding (8 NeuronCores): 2 batch groups x 4-way head/tensor parallel.
  core c: g = c//4 (batch), li = c%4 -> heads [4*li, 4*li+4)
  - QKV projections + attention for its 4 heads over the full 2048-token seq
  - partial o-proj (its 256 ctx dims) -> ReduceScatter(add) over the 4-core
    group, each core keeping tokens [512*li, 512*(li+1))
  - LN1 + FFN (bf16 weights) + LN2 on its 512-token slice
All matmuls run bf16 (f32 PSUM accumulate); weights are pre-transposed
host-side into partition-major HBM layouts (one contiguous DMA block per
partition). o-proj/store/RS/LN1 work is software-pipelined into the next
query block's inner loop; LayerNorm rstd uses exp(-0.5*ln(var+eps)) so the
scalar engine never reloads its activation table inside attention.
"""
import sys, os
sys.path.insert(0, '/opt/trn_rl_repo')
import numpy as np
import ml_dtypes
import concourse.bass as bass
import concourse.tile as tile
from concourse import bacc, mybir
from concourse.bass_utils import run_bass_kernel_spmd
from concourse.masks import make_identity

F32 = mybir.dt.float32
F32R = mybir.dt.float32r
BF16 = mybir.dt.bfloat16
AF = mybir.ActivationFunctionType
OP = mybir.AluOpType

S, D, HD, F = 2048, 1024, 64, 4096
GROUPS = [[0, 1, 2, 3], [4, 5, 6, 7]]
EPS = 1e-12

_CACHE = {}


def _bcast_ap(ap, p=128):
    return bass.AP(tensor=ap.tensor, offset=ap.offset, ap=[[0, p]] + list(ap.ap))


def _build(nrep=1):
    nc = bacc.Bacc("TRN2", target_bir_lowering=False, debug=False, num_devices=8)

    # weights arrive pre-transposed from _in_maps into partition-major
    # layouts so every DMA is one contiguous block per partition (the old
    # "(a p) f" rearranges cost ~160ns/descriptor of SEQ issue time)
    xg = nc.dram_tensor("xg", [S, D], F32, kind="ExternalInput").ap()
    wq = nc.dram_tensor("wq", [128, 2048], BF16, kind="ExternalInput").ap()
    wk = nc.dram_tensor("wk", [128, 2048], BF16, kind="ExternalInput").ap()
    wv = nc.dram_tensor("wv", [128, 2048], BF16, kind="ExternalInput").ap()
    wo = nc.dram_tensor("wo", [256, D], BF16, kind="ExternalInput").ap()
    w1 = nc.dram_tensor("w1", [128, 4, 8192], BF16, kind="ExternalInput").ap()
    w2 = nc.dram_tensor("w2", [128, 4, 8192], BF16, kind="ExternalInput").ap()
    bq = nc.dram_tensor("bq", [128, 2], F32, kind="ExternalInput").ap()
    bk = nc.dram_tensor("bk", [128, 2], F32, kind="ExternalInput").ap()
    bv = nc.dram_tensor("bv", [256], F32, kind="ExternalInput").ap()
    bo = nc.dram_tensor("bo", [D], F32, kind="ExternalInput").ap()
    b1 = nc.dram_tensor("b1", [128, 32], F32, kind="ExternalInput").ap()
    b2 = nc.dram_tensor("b2", [D], F32, kind="ExternalInput").ap()
    ln1g = nc.dram_tensor("ln1g", [D], F32, kind="ExternalInput").ap()
    ln1b = nc.dram_tensor("ln1b", [D], F32, kind="ExternalInput").ap()
    ln2g = nc.dram_tensor("ln2g", [D], F32, kind="ExternalInput").ap()
    ln2b = nc.dram_tensor("ln2b", [D], F32, kind="ExternalInput").ap()
    out = nc.dram_tensor("out", [512, D], F32, kind="ExternalOutput").ap()

    rs_in = nc.dram_tensor("rs_in", [S, D], F32)
    rs_out = nc.dram_tensor("rs_out", [512, D], F32)

    t = locals()
    with tile.TileContext(nc) as tc:
        for _r in range(nrep):
            _emit(nc, tc, t)
    nc.compile()
    return nc


def _emit(nc, tc, t):
    from contextlib import ExitStack
    from itertools import cycle
    PH = os.environ.get("BGE_KERNEL_PHASES", "full")
    xg, wq, wk, wv, wo, w1, w2 = t["xg"], t["wq"], t["wk"], t["wv"], t["wo"], t["w1"], t["w2"]
    bq, bk, bv, bo, b1, b2 = t["bq"], t["bk"], t["bv"], t["bo"], t["b1"], t["b2"]
    ln1g, ln1b, ln2g, ln2b = t["ln1g"], t["ln1b"], t["ln2g"], t["ln2b"]
    out, rs_in, rs_out = t["out"], t["rs_in"], t["rs_out"]

    with ExitStack() as top:
        const = top.enter_context(tc.tile_pool(name="const", bufs=1))
        stp = top.enter_context(tc.tile_pool(name="stp", bufs=2))

        ident = const.tile([128, 128], F32)
        make_identity(nc, ident[:])
        eps = const.tile([128, 1], F32)
        nc.vector.memset(eps[:], EPS)
        ones1f = const.tile([1, 64], F32)
        nc.vector.memset(ones1f[:], 1.0)
        ones1 = const.tile([1, 64], F32R)
        nc.vector.tensor_copy(ones1[:], ones1f[:])
        onesc = const.tile([128, 4, 1], F32)
        nc.vector.memset(onesc[:], 1.0)

        def bc_tile(src, n, name, pool=None):
            tl = (pool or const).tile([128, n], F32, name=name)
            nc.gpsimd.dma_start(out=tl[:], in_=_bcast_ap(src))
            return tl

        bv_b = bc_tile(bv, 256, "bv_b")
        lnp = top.enter_context(tc.tile_pool(name="lnp", bufs=1))
        A_t = [lnp.tile([128, D], F32, name=f"a{i}") for i in range(4)]

        def layernorm2p(dst, src, g_b, be_b):
            """LN with apply passes split across DVE (cols 0:640) and GpSimd (640:1024)."""
            stats = stp.tile([128, 2, 6], F32, name="stats")
            for sgi in range(2):
                nc.vector.bn_stats(out=stats[:, sgi, :], in_=src[:, sgi * 512:(sgi + 1) * 512])
            mv = stp.tile([128, 2], F32, name="mv")
            nc.vector.bn_aggr(out=mv[:], in_=stats[:])
            rstd = stp.tile([128, 1], F32, name="rstd")
            # rstd = (var+eps)^-0.5 via ln+exp: both live in the SAME act
            # table as the softmax exp, so no table reload (1283ns each)
            # and no DVE reciprocal on the LN critical path
            nc.scalar.activation(out=rstd[:], in_=mv[:, 1:2], func=AF.Ln,
                                 bias=eps[:], scale=1.0)
            nc.scalar.activation(out=rstd[:], in_=rstd[:], func=AF.Exp,
                                 bias=0.0, scale=-0.5)
            for eng, c0, c1 in ((nc.vector, 0, 640), (nc.gpsimd, 640, 1024)):
                eng.tensor_scalar(out=dst[:, c0:c1], in0=src[:, c0:c1],
                                  scalar1=mv[:, 0:1], scalar2=rstd[:],
                                  op0=OP.subtract, op1=OP.mult)
                eng.tensor_tensor(out=dst[:, c0:c1], in0=dst[:, c0:c1],
                                  in1=g_b[:, c0:c1], op=OP.mult)
                eng.tensor_tensor(out=dst[:, c0:c1], in0=dst[:, c0:c1],
                                  in1=be_b[:, c0:c1], op=OP.add)

        def layernorm(dst, src, g_b, be_b):
            stats = stp.tile([128, 2, 6], F32, name="stats")
            for sgi in range(2):
                nc.vector.bn_stats(out=stats[:, sgi, :], in_=src[:, sgi * 512:(sgi + 1) * 512])
            mv = stp.tile([128, 2], F32, name="mv")
            nc.vector.bn_aggr(out=mv[:], in_=stats[:])
            rstd = stp.tile([128, 1], F32, name="rstd")
            nc.scalar.activation(out=rstd[:], in_=mv[:, 1:2], func=AF.Ln,
                                 bias=eps[:], scale=1.0)
            nc.scalar.activation(out=rstd[:], in_=rstd[:], func=AF.Exp,
                                 bias=0.0, scale=-0.5)
            nc.vector.tensor_scalar(out=dst[:], in0=src[:], scalar1=mv[:, 0:1],
                                    scalar2=rstd[:], op0=OP.subtract, op1=OP.mult)
            nc.vector.tensor_mul(out=dst[:], in0=dst[:], in1=g_b[:])
            nc.vector.tensor_add(out=dst[:], in0=dst[:], in1=be_b[:])
        b1_sb = const.tile([128, 32], F32, name="b1_sb")
        nc.gpsimd.dma_start(out=b1_sb[:], in_=b1)
        bq_sb = const.tile([128, 2], F32, name="bq_sb")
        nc.gpsimd.dma_start(out=bq_sb[:], in_=bq)
        bk_sb = const.tile([128, 2], F32, name="bk_sb")
        nc.gpsimd.dma_start(out=bk_sb[:], in_=bk)

        with ExitStack() as ao_stack:
          octx = ao_stack.enter_context(tc.tile_pool(name="octx", bufs=1))
          Ctx = [octx.tile([128, S], BF16, name=f"ctx{i}") for i in range(2)]
          wo_t = octx.tile([128, 2, D], BF16, name="wo_t")
          for dc2 in range(2):
              (nc.gpsimd, nc.sync)[dc2].dma_start(
                  out=wo_t[:, dc2, :], in_=wo[dc2 * 128:(dc2 + 1) * 128, :])
          with ExitStack() as att_stack:
              attp = att_stack.enter_context(tc.tile_pool(name="attp", bufs=1))
              # bf16 Q/K: HW runs the 64-contraction f32r matmul at half the
              # modeled rate (443ns vs 213ns measured); bf16 is full rate
              Qt = [attp.tile([128, S], BF16, name=f"qt{i}") for i in range(2)]
              Kt = [attp.tile([128, S], BF16, name=f"kt{i}") for i in range(2)]
              Vaug = [attp.tile([128, 4, 65], BF16, name=f"va{kc}") for kc in range(16)]

              # ------------- Phase P: transpose x + QKV projections -------------
              with ExitStack() as ph:
                  psA = ph.enter_context(tc.tile_pool(name="psP", bufs=4, space="PSUM"))
                  xap = ph.enter_context(tc.tile_pool(name="xap", bufs=3))
                  xtp = ph.enter_context(tc.tile_pool(name="xtp", bufs=1))
                  wp = ph.enter_context(tc.tile_pool(name="wp", bufs=1))
                  wq_t = wp.tile([128, 8, 256], BF16, name="wq_t")
                  wk_t = wp.tile([128, 8, 256], BF16, name="wk_t")
                  wv_t = wp.tile([128, 8, 256], BF16, name="wv_t")
                  _xengs = cycle((nc.sync, nc.gpsimd, nc.scalar))

                  def load_xa(ts):
                      xa = xap.tile([128, 4, D], F32, name="xa")
                      for tc4 in range(4):
                          nstrip = 4 if ts == 0 else 2
                          w = 1024 // nstrip
                          for hh in range(nstrip):
                              next(_xengs).dma_start(
                                  out=xa[:, tc4, hh * w:(hh + 1) * w],
                                  in_=xg[ts * 512 + tc4 * 128:
                                         ts * 512 + (tc4 + 1) * 128,
                                         hh * w:(hh + 1) * w])
                      return xa

                  xas = {0: load_xa(0)}
                  _wengs = cycle((nc.gpsimd, nc.sync, nc.scalar))
                  for wi, (_wt, _w) in enumerate(((wk_t, wk), (wq_t, wq), (wv_t, wv))):
                      for g4 in range(2):
                          next(_wengs).dma_start(
                              out=_wt[:, g4 * 4:(g4 + 1) * 4, :],
                              in_=_w[:, g4 * 1024:(g4 + 1) * 1024])
                      xas[wi + 1] = load_xa(wi + 1)

                  for ts in range(4):
                      xa = xas[ts]
                      Xts = [xtp.tile([128, 512], BF16, name=f"xt{dc}") for dc in range(8)]
                      for tc4 in range(4):
                          for dc in range(8):
                              pt = psA.tile([128, 128], F32, name="ps")
                              nc.tensor.transpose(pt[:], xa[:, tc4, dc * 128:(dc + 1) * 128], ident[:])
                              nc.vector.tensor_copy(Xts[dc][:, tc4 * 128:(tc4 + 1) * 128], pt[:])
                      for w_t, b_sb, Dst in ((wk_t, bk_sb, Kt), (wq_t, bq_sb, Qt)):
                          for oc in range(2):
                              pk = psA.tile([128, 512], F32, name="ps")
                              for dc in range(8):
                                  nc.tensor.matmul(pk[:], w_t[:, dc, oc * 128:(oc + 1) * 128],
                                                   Xts[dc][:], start=(dc == 0), stop=(dc == 7))
                              nc.vector.tensor_scalar_add(
                                  out=Dst[oc][:, ts * 512:(ts + 1) * 512], in0=pk[:],
                                  scalar1=b_sb[:, oc:oc + 1])
                      for tc4 in range(4):
                          kc = ts * 4 + tc4
                          pv = psA.tile([128, 256], F32, name="ps")
                          for dc in range(8):
                              nc.tensor.matmul(pv[:], Xts[dc][:, tc4 * 128:(tc4 + 1) * 128],
                                               wv_t[:, dc, :], start=(dc == 0), stop=(dc == 7))
                          nc.vector.tensor_tensor(
                              out=Vaug[kc][:, :, 0:64],
                              in0=pv[:].rearrange("p (h d) -> p h d", h=4),
                              in1=bv_b[:].rearrange("p (h d) -> p h d", h=4),
                              op=OP.add)
                          nc.vector.tensor_copy(Vaug[kc][:, :, 64:65], onesc[:])

              # ---- Phase A+O: attention interleaved with o-proj + chunked RS ----
              with ExitStack() as ph:
                if PH in ("pa", "pao", "paor", "paof", "full"):
                  expp = ph.enter_context(tc.tile_pool(name="expp", bufs=2))
                  rzp = ph.enter_context(tc.tile_pool(name="rzp", bufs=2))
                  stgp = ph.enter_context(tc.tile_pool(name="stgp", bufs=1))
                  scP = ph.enter_context(tc.tile_pool(name="scP", bufs=2, space="PSUM"))
                  psO = ph.enter_context(tc.tile_pool(name="psO", bufs=1, space="PSUM"))
                  psB = ph.enter_context(tc.tile_pool(name="psB", bufs=2, space="PSUM"))
                  psC = ph.enter_context(tc.tile_pool(name="psC", bufs=1, space="PSUM"))
                  lnc1 = ph.enter_context(tc.tile_pool(name="lnc1", bufs=1))
                  rawp = ph.enter_context(tc.tile_pool(name="rawp", bufs=2))
                  do_o = PH in ("pao", "paor", "paof", "full")
                  do_rs = PH in ("paor", "full")
                  bo_b = bc_tile(bo, D, "bo_b", lnc1)
                  ln1g_b = bc_tile(ln1g, D, "ln1g_b", lnc1)
                  ln1b_b = bc_tile(ln1b, D, "ln1b_b", lnc1)
                  from collections import deque
                  pending = deque()

                  def queue_oproj(qb):
                      """Defer qb's o-proj/store/RS/LN1 as work items that the
                      NEXT qb's inner loop drains two-at-a-time, so the PE
                      queue never bursts 16 o-proj matmuls while Act starves."""
                      if PH not in ("pao", "paor", "paof", "full"):
                          return
                      sA = stgp.tile([128, 4, D], F32, name="sA")

                      def mk_mm(q4, oh):
                          def go():
                              po = psO.tile([128, 512], F32, name="po")
                              tc16 = qb * 4 + q4
                              for dc2 in range(2):
                                  nc.tensor.matmul(
                                      po[:], Ctx[dc2][:, tc16 * 128:(tc16 + 1) * 128],
                                      wo_t[:, dc2, oh * 512:(oh + 1) * 512],
                                      start=(dc2 == 0), stop=(dc2 == 1))
                              nc.vector.tensor_copy(
                                  sA[:, q4, oh * 512:(oh + 1) * 512], po[:])
                          return go

                      def mk_store(q4):
                          def go():
                              (nc.sync, nc.gpsimd, nc.scalar, nc.sync)[q4].dma_start(
                                  out=rs_in[(qb * 4 + q4) * 128:(qb * 4 + q4 + 1) * 128, :],
                                  in_=sA[:, q4, :])
                          return go

                      for q4 in range(4):
                          for oh in range(2):
                              pending.append(mk_mm(q4, oh))
                          pending.append(mk_store(q4))

                      def tail():
                          if PH in ("paor", "full"):
                              nc.gpsimd.collective_compute(
                                  "ReduceScatter", OP.add,
                                  ins=[rs_in[qb * 512:(qb + 1) * 512, :]],
                                  outs=[rs_out[qb * 128:(qb + 1) * 128, :]],
                                  replica_groups=GROUPS)
                          if PH in ("paor", "paof", "full"):
                              raw = rawp.tile([128, D], F32, name="raw")
                              for hh in range(2):
                                  (nc.sync, nc.gpsimd)[hh].dma_start(
                                      out=raw[:, hh * 512:(hh + 1) * 512],
                                      in_=rs_out[qb * 128:(qb + 1) * 128,
                                                 hh * 512:(hh + 1) * 512])
                              nc.vector.tensor_add(out=raw[:], in0=raw[:], in1=bo_b[:])
                              layernorm(A_t[qb], raw, ln1g_b, ln1b_b)
                      pending.append(tail)

                  def drain(n):
                      for _ in range(n):
                          if pending:
                              pending.popleft()()

                  for qb in range(4):
                      for hp in range(2):
                          avs = [psB.tile([65, 512], F32, name="av") for i in range(2)]
                          for kp in range(8):
                              for i in range(2):
                                  sc = scP.tile([128, 1024], F32, name="sc2")
                                  for half in range(2):
                                      kc = 2 * kp + half
                                      nc.tensor.matmul(
                                          sc[:, half * 512:(half + 1) * 512],
                                          Kt[hp][i * 64:(i + 1) * 64, kc * 128:(kc + 1) * 128],
                                          Qt[hp][i * 64:(i + 1) * 64, qb * 512:(qb + 1) * 512],
                                          start=True, stop=True)
                                  e = expp.tile([128, 1024], BF16, name=f"e{i}")
                                  nc.scalar.activation(e[:], sc[:], AF.Exp)
                                  for half in range(2):
                                      kc = 2 * kp + half
                                      nc.tensor.matmul(
                                          avs[i][:], Vaug[kc][:, 2 * hp + i, :],
                                          e[:, half * 512:(half + 1) * 512],
                                          start=(kc == 0), stop=(kc == 15))
                              drain(2)
                          for i in range(2):
                              rz = rzp.tile([1, 512], F32R, name="rz")
                              with nc.allow_low_precision(reason="f32r is full width"):
                                  nc.vector.reciprocal(rz[:], avs[i][64:65, :])
                              bcp = psC.tile([64, 512], F32, name="bcp")
                              nc.tensor.matmul(bcp[:], ones1[:], rz[:], start=True, stop=True)
                              rzs = rzp.tile([64, 512], F32, name="rzs")
                              nc.vector.tensor_copy(rzs[:], bcp[:])
                              nc.vector.tensor_mul(
                                  out=Ctx[hp][i * 64:(i + 1) * 64, qb * 512:(qb + 1) * 512],
                                  in0=avs[i][0:64, :], in1=rzs[:])
                      queue_oproj(qb)
                  while pending:
                      pending.popleft()()

        # ------------- Phase F: LN1 + FFN + LN2 -------------
        if PH not in ("full", "paof"):
            return
        with ExitStack() as ph:
            lnc2 = ph.enter_context(tc.tile_pool(name="lnc2", bufs=1))
            b2_b = bc_tile(b2, D, "b2_b", lnc2)
            ln2g_b = bc_tile(ln2g, D, "ln2g_b", lnc2)
            ln2b_b = bc_tile(ln2b, D, "ln2b_b", lnc2)
            sbA = ph.enter_context(tc.tile_pool(name="sbA", bufs=1))
            w1p = ph.enter_context(tc.tile_pool(name="w1p", bufs=2))
            w2p = ph.enter_context(tc.tile_pool(name="w2p", bufs=2))
            hp_ = ph.enter_context(tc.tile_pool(name="hp", bufs=2))
            fmisc = ph.enter_context(tc.tile_pool(name="fmisc", bufs=2))
            psA = ph.enter_context(tc.tile_pool(name="psF", bufs=4, space="PSUM"))
            psD = ph.enter_context(tc.tile_pool(name="psD", bufs=4, space="PSUM"))

            At = [sbA.tile([128, 512], BF16, name=f"at{dc}") for dc in range(8)]
            for tc4 in range(4):
                for dc in range(8):
                    pt = psA.tile([128, 128], F32, name="ps")
                    nc.tensor.transpose(pt[:], A_t[tc4][:, dc * 128:(dc + 1) * 128], ident[:])
                    nc.scalar.copy(At[dc][:, tc4 * 128:(tc4 + 1) * 128], pt[:])

            ffn_acc = [sbA.tile([128, D], F32, name=f"fa{i}") for i in range(4)]
            oall = sbA.tile([128, 4, D], F32, name="oall")
            for tc4 in range(4):
                nc.vector.tensor_add(out=ffn_acc[tc4][:], in0=A_t[tc4][:], in1=b2_b[:])
            for fg in range(4):
                w1t = w1p.tile([128, 8, 1024], BF16, name="w1t")
                w2t = w2p.tile([128, 8, D], BF16, name="w2t")
                _fengs = cycle((nc.gpsimd, nc.sync, nc.scalar))
                for g4 in range(4):
                    next(_fengs).dma_start(
                        out=w1t[:, g4 * 2:(g4 + 1) * 2, :],
                        in_=w1[:, fg:fg + 1, g4 * 2048:(g4 + 1) * 2048])
                for g4 in range(4):
                    next(_fengs).dma_start(
                        out=w2t[:, g4 * 2:(g4 + 1) * 2, :],
                        in_=w2[:, fg:fg + 1, g4 * 2048:(g4 + 1) * 2048])
                hts = [hp_.tile([128, 512], BF16, name=f"h{fc}") for fc in range(8)]
                for fc8 in range(8):
                    phm = psA.tile([128, 512], F32, name="ps")
                    for dc in range(8):
                        nc.tensor.matmul(phm[:], w1t[:, dc, fc8 * 128:(fc8 + 1) * 128],
                                         At[dc][:], start=(dc == 0), stop=(dc == 7))
                    fci = fg * 8 + fc8
                    tmp = fmisc.tile([128, 512], F32, name="tmp")
                    nc.vector.tensor_scalar(out=tmp[:], in0=phm[:],
                                            scalar1=b1_sb[:, fci:fci + 1], scalar2=0.0,
                                            op0=OP.add, op1=OP.max)
                    nc.scalar.activation(hts[fc8][:], tmp[:], AF.Gelu)
                if fg < 3:
                    for oh in range(2):
                        paccs = [psD.tile([128, 512], F32, name="pac") for i in range(4)]
                        for fc8 in range(8):
                            for tc4 in range(4):
                                nc.tensor.matmul(paccs[tc4][:],
                                                 hts[fc8][:, tc4 * 128:(tc4 + 1) * 128],
                                                 w2t[:, fc8, oh * 512:(oh + 1) * 512],
                                                 start=(fc8 == 0), stop=(fc8 == 7))
                        for tc4 in range(4):
                            dst = ffn_acc[tc4][:, oh * 512:(oh + 1) * 512]
                            nc.vector.tensor_add(out=dst, in0=dst, in1=paccs[tc4][:])
                else:
                    # last group tc4-major: LN2(tc4) overlaps fc2 of tc4+1
                    for tc4 in range(4):
                        paccs = [psD.tile([128, 512], F32, name="pac") for i in range(2)]
                        for oh in range(2):
                            for fc8 in range(8):
                                nc.tensor.matmul(paccs[oh][:],
                                                 hts[fc8][:, tc4 * 128:(tc4 + 1) * 128],
                                                 w2t[:, fc8, oh * 512:(oh + 1) * 512],
                                                 start=(fc8 == 0), stop=(fc8 == 7))
                        acc = ffn_acc[tc4]
                        for oh in range(2):
                            dst = acc[:, oh * 512:(oh + 1) * 512]
                            nc.vector.tensor_add(out=dst, in0=dst, in1=paccs[oh][:])
                        layernorm2p(oall[:, tc4, :], acc, ln2g_b, ln2b_b)
                        for s4 in range(4):
                            (nc.sync, nc.gpsimd, nc.scalar, nc.sync)[s4].dma_start(
                                out=out[tc4 * 128:(tc4 + 1) * 128,
                                        s4 * 256:(s4 + 1) * 256],
                                in_=oall[:, tc4, s4 * 256:(s4 + 1) * 256])


def _get_nc():
    if "nc" not in _CACHE:
        _CACHE["nc"] = _build()
    return _CACHE["nc"]


def _qkv_pmajor(w):
    # [1024, 256] -> [128, 2048]: w_t[p, g4*1024 + a*256 + f] = w[g4*512+a*128+p, f]
    return np.ascontiguousarray(
        w.reshape(2, 4, 128, 256).transpose(2, 0, 1, 3).reshape(128, 2048))


def _w1_pmajor(w1):
    # [1024, 4096] -> [128, 4, 8192]:
    # w1n[p, fg, g4*2048 + a*1024 + f] = w1[g4*256 + a*128 + p, fg*1024 + f]
    return np.ascontiguousarray(
        w1.reshape(4, 2, 128, 4, 1024).transpose(2, 3, 0, 1, 4).reshape(128, 4, 8192))


def _w2_pmajor(w2):
    # [4096, 1024] -> [128, 4, 8192]:
    # w2n[p, fg, g4*2048 + a*1024 + f] = w2[fg*1024 + g4*256 + a*128 + p, f]
    return np.ascontiguousarray(
        w2.reshape(4, 4, 2, 128, 1024).transpose(3, 0, 1, 2, 4).reshape(128, 4, 8192))


def _in_maps(inputs):
    x = np.asarray(inputs["x"], dtype=np.float32)
    w1n = _w1_pmajor(np.asarray(inputs["W1"], np.float32)).astype(ml_dtypes.bfloat16)
    w2n = _w2_pmajor(np.asarray(inputs["W2"], np.float32)).astype(ml_dtypes.bfloat16)
    b1n = np.ascontiguousarray(
        np.asarray(inputs["b1"], np.float32).reshape(32, 128).T)
    maps = []
    for c in range(8):
        g, li = c // 4, c % 4
        cs = slice(256 * li, 256 * (li + 1))
        m = {
            "xg": np.ascontiguousarray(x[g]),
            "wq": _qkv_pmajor(np.asarray(inputs["Wq"], np.float32)[:, cs] / 8.0).astype(ml_dtypes.bfloat16),
            "wk": _qkv_pmajor(np.asarray(inputs["Wk"], np.float32)[:, cs]).astype(ml_dtypes.bfloat16),
            "wv": _qkv_pmajor(np.asarray(inputs["Wv"], np.float32)[:, cs]).astype(ml_dtypes.bfloat16),
            "wo": np.ascontiguousarray(np.asarray(inputs["Wo"], np.float32)[cs, :]).astype(ml_dtypes.bfloat16),
            "w1": w1n,
            "w2": w2n,
            "bq": np.ascontiguousarray(
                np.asarray(inputs["bq"], np.float32)[cs].reshape(2, 128).T) / 8.0,
            "bk": np.ascontiguousarray(
                np.asarray(inputs["bk"], np.float32)[cs].reshape(2, 128).T),
            "bv": np.ascontiguousarray(np.asarray(inputs["bv"], np.float32)[cs]),
            "bo": np.asarray(inputs["bo"], np.float32),
            "b1": np.asarray(inputs["b1"], np.float32),
            "b2": np.asarray(inputs["b2"], np.float32),
            "ln1g": np.asarray(inputs["ln1_g"], np.float32),
            "ln1b": np.asarray(inputs["ln1_b"], np.float32),
            "ln2g": np.asarray(inputs["ln2_g"], np.float32),
            "ln2b": np.asarray(inputs["ln2_b"], np.float32),
        }
        maps.append(m)
    return maps


def run(inputs, trace=False):
    nc = _get_nc()
    res = run_bass_kernel_spmd(nc, _in_maps(inputs), list(range(8)), trace=trace)
    B = 2
    full = np.empty((B, S, D), np.float32)
    for c in range(8):
        g, li = c // 4, c % 4
        o = res.results[c]["out"]
        for j in range(4):
            full[g, j * 512 + li * 128: j * 512 + (li + 1) * 128, :] = \
                o[j * 128:(j + 1) * 128]
    return full, res


def kernel(**inputs):
    return run(inputs)[0]

